# revision 46
# baseline (speedup 1.0000x reference)
"""LoRA QKV fused projection kernel for 8 TRN2 NeuronCores.

Reference computation (T=8192 tokens, HID=4096, D=6144 out, S=8 slots, R=16):
    y = x @ W.T
    a[t,s,i,r] = sum_h x[t,h] * lora_A[s,i,r,h]         (down-proj, all slots)
    a *= onehot(token_to_slot)[t,s] * scaling[s]         (routing gate)
    d[t, :] = concat_i( sum_{s,r} a[t,s,i,r] * B_i[s,:,r] )   (up-proj)
    out = y + d

Sharding: pure token-DP — core c owns tokens [c*1024, (c+1)*1024) with the
full hidden and output dims. Its x shard (8.4 MB bf16) is loaded to SBUF once
as the main-GEMM moving operand; W streams through once (50 MB bf16/core).

Numerics/throughput choices (error budget: harness gate is 2e-2, we land
~1.68e-2, measured bit-stable across runs on the fixed-seed inputs):
  * main GEMM inputs bf16 (fp32 PSUM accumulate): bf16 enables the fast
    weight load path (fp32/fp32r cannot FWL), so the per-matmul stationary
    reload (~97 ns) hides under the previous matmul's 512-column stream in
    the PE's 64-deep reorder window. Measured back-to-back cadence: 216 ns.
  * LoRA down-proj AND 6 of the main GEMM's 32 k-tiles in fp8e4m3 with
    perf_mode=DoubleRow (2 k-tiles packed per PE cell, 2x MACs/cycle). A
    DoubleRow matmul costs ~213 ns (256-column LDWEIGHTS-bound, FWL off)
    but replaces TWO 216 ns bf16 matmuls. DR and bf16 matmuls are kept in
    contiguous blocks — alternating modes costs ~45 ns/switch (measured).
    Scale plan: x8 = x/4 and w8 = 4W (product exact in PSUM, no rescale
    hook exists); a8 = 16*lora_A with the 64x folded into the gate. The
    scales lift the 0.02-std weights out of e4m3's subnormal range.
  * LoRA up-proj fused into the main GEMM's PSUM accumulation: per output
    row-block mb, after the k-tile matmuls, one extra matmul with B as
    stationary (contracting the 128 (slot,rank) pairs of the gated
    down-proj activations) lands the delta in the same PSUM bank
    (start=False). Single drain emits final rows — no partials, no host
    reduce.

DMA plan: SP HWDGE ring carries w0,w1 -> x(bf16) -> w2..w47; ACT ring
carries the phase-A fp8 stream (x8/a8 chunks) -> gate -> B -> output drains.
"""

import numpy as np

# problem shape (hardcoded per harness contract)
T = 8192
HID = 4096
Q_SIZE = 4096
KV_SIZE = 1024
D = Q_SIZE + 2 * KV_SIZE  # 6144
S = 8
R = 16
NCORES = 8
P = 128

TC = T // NCORES          # 1024 tokens per core
KA = HID // P             # 32 k-tiles
K2 = KA // 2              # 16 double-row k-tile pairs
MB = D // P               # 48 output row-blocks of 128
NH = TC // 512            # 2 moving n-halves of 512 tokens
M8 = 3                    # k-tile PAIRS of the main GEMM done in fp8 DoubleRow
KBF = KA - 2 * M8         # remaining bf16 k-tiles of the main GEMM
# fp8 scale plan: products must come out exact in PSUM (no rescale hook), so
# scale(x8)*scale(W8) = 1; A's scale is compensated in the gate. The scales
# lift W (std 0.02) out of e4m3's subnormal range (<2^-6) while keeping x
# mostly normal.
XS8 = 0.25                # x8 = XS8 * x
AS8 = 16.0                # a8 = AS8 * lora_A
WS8 = 1.0 / XS8           # w8 = WS8 * W  (fp8 k-slice)

# phase-A chunk sizes in k-tile-pair units (tiny first chunks => first
# matmul waits on ~0.36 MB only). One SBUF buffer per chunk — a recycled
# buffer would make a later chunk's DMA wait on earlier matmuls, blocking
# the whole SP ring behind it (head-of-line).
CHUNKS2 = [1, 1, 1, 1, 2, 2, 2, 2, 2, 2]
assert sum(CHUNKS2) == K2
# bf16 x chunks on the SP ring (coarse; only consumed from phase C on; a
# smaller first chunk so mb0 can start sooner)
CHUNKS_X = [6, 10, 10]
assert sum(CHUNKS_X) == KBF

_CACHE = {}


def _build_nc():
    import concourse.mybir as mybir
    import concourse.tile as tile
    from concourse import bacc

    bf16 = mybir.dt.bfloat16
    fp8 = mybir.dt.float8e4
    f32 = mybir.dt.float32
    COPY = mybir.ActivationFunctionType.Copy
    DR = mybir.MatmulPerfMode.DoubleRow

    nc = bacc.Bacc(None, target_bir_lowering=False, debug=False)

    # ---- DRAM parameters (per-core shapes; declaration order = binding order)
    x_d = nc.declare_dram_parameter("x_t", [P, KBF, TC], bf16, isOutput=False)
    x8_d = nc.declare_dram_parameter("x8_t", [P, K2, 2, TC], fp8, isOutput=False)
    a8_d = nc.declare_dram_parameter("a8_t", [P, K2, 2, 3, P], fp8, isOutput=False)
    w_d = nc.declare_dram_parameter("w_t", [MB, P, KBF, P], bf16, isOutput=False)
    w8_d = nc.declare_dram_parameter("w8_t", [MB, P, M8, 2, P], fp8, isOutput=False)
    b_d = nc.declare_dram_parameter("b_t", [P, MB, P], bf16, isOutput=False)
    g_d = nc.declare_dram_parameter("gate", [P, TC], f32, isOutput=False)
    y_d = nc.declare_dram_parameter("y", [MB, P, TC], f32, isOutput=True)

    with tile.TileContext(nc) as tc:
        with tc.tile_pool(name="xres", bufs=1) as xres_pool, \
             tc.tile_pool(name="wp", bufs=3) as w_pool, \
             tc.tile_pool(name="x8p", bufs=10) as x8_pool, \
             tc.tile_pool(name="a8p", bufs=10) as a8_pool, \
             tc.tile_pool(name="agp", bufs=1) as ag_pool, \
             tc.tile_pool(name="bp", bufs=1) as b_pool, \
             tc.tile_pool(name="stp", bufs=4) as st_pool, \
             tc.tile_pool(name="psum", bufs=8, space="PSUM") as ps_pool:

            # resident bf16 moving operand for the main GEMM (k-tiles 4..31;
            # k-tiles 0..3 live in the pinned fp8 x8 chunks 0 and 1)
            x_res = xres_pool.tile([P, KBF, TC], bf16, tag="xres")

            # ---------------- Phase A: LoRA down-proj aT = A @ x (fp8) ------
            # aT[(i,sr), t] accumulated in 6 psum banks over 16 DoubleRow
            # k-pair steps, chasing the chunked x8/a8 loads. The fp8 stream
            # gets a clean prefix on the SP ring (critical early data); the
            # bulk bf16 x / W stream follows it.
            ps_a = [
                ps_pool.tile([P, 512], f32, tag="ps", name=f"ps_a{i}_{h}")
                for i in range(3) for h in range(2)
            ]
            k0s = [sum(CHUNKS2[:c]) for c in range(len(CHUNKS2))]
            ab_tiles = []
            for ch, (k0, cw) in enumerate(zip(k0s, CHUNKS2)):
                ksl = slice(k0, k0 + cw)
                x8_t = x8_pool.tile([P, cw, 2, TC], fp8, tag="x8", name=f"x8{ch}")
                nc.sync.dma_start(out=x8_t[:], in_=x8_d[:, ksl, :, :])
                a8_t = a8_pool.tile([P, cw, 2, 3, P], fp8, tag="a8", name=f"a8{ch}")
                nc.scalar.dma_start(out=a8_t[:], in_=a8_d[:, ksl, :, :, :])
                ab_tiles.append((x8_t, a8_t))
            # SP ring continues: x_res, then the W stream from mb2 on; the
            # last x chunk rides the ACT ring (behind b) to balance fronts
            xk0 = 0
            for ci, cw in enumerate(CHUNKS_X):
                ksl = slice(xk0, xk0 + cw)
                xq = nc.scalar if ci == len(CHUNKS_X) - 1 else nc.sync
                xq.dma_start(out=x_res[:, ksl, :], in_=x_d[:, ksl, :])
                xk0 += cw
            gate_t = ag_pool.tile([P, TC], f32, tag="gate")
            nc.scalar.dma_start(out=gate_t[:], in_=g_d[:])
            # first two W tiles ride the quiet ACT ring: the SP ring's front
            # (x8 + x_res) is the saturation bottleneck, and these aren't
            # needed until mb0/mb1 (~31/45 us)
            w_tiles = []
            for mb in range(2):
                w_t = w_pool.tile([P, KBF, P], bf16, tag="w", name=f"w{mb}")
                nc.scalar.dma_start(out=w_t[:], in_=w_d[mb])
                w8_t = w_pool.tile([P, M8, 2, P], fp8, tag="w8", name=f"w8_{mb}")
                nc.scalar.dma_start(out=w8_t[:], in_=w8_d[mb])
                w_tiles.append((w_t, w8_t))
            b_t = b_pool.tile([P, MB, P], bf16, tag="b")
            nc.scalar.dma_start(out=b_t[:], in_=b_d[:])

            for ch, (k0, cw) in enumerate(zip(k0s, CHUNKS2)):
                x8_t, a8_t = ab_tiles[ch]
                for kk in range(cw):
                    first = k0 + kk == 0
                    last = k0 + kk == K2 - 1
                    for i in range(3):
                        for h in range(2):
                            nc.tensor.matmul(
                                ps_a[i * 2 + h][:],
                                a8_t[:, kk, :, i, :],
                                x8_t[:, kk, :, h * 512:(h + 1) * 512],
                                start=first, stop=last,
                                perf_mode=DR,
                            )

            # ---------------- Phase B: routing gate -------------------------
            ag = []
            for i in range(3):
                ag_t = ag_pool.tile([P, TC], bf16, tag=f"ag{i}", name=f"ag{i}")
                for h in range(2):
                    sl = slice(h * 512, (h + 1) * 512)
                    nc.vector.tensor_mul(ag_t[:, sl], ps_a[i * 2 + h][:], gate_t[:, sl])
                ag.append(ag_t)

            # ---------------- Phase C: main GEMM + fused LoRA up-proj -------
            for mb in range(MB):
                if mb < 2:
                    w_t, w8_t = w_tiles[mb]
                else:
                    w_t = w_pool.tile([P, KBF, P], bf16, tag="w", name=f"w{mb}")
                    nc.sync.dma_start(out=w_t[:], in_=w_d[mb])
                    w8_t = w_pool.tile([P, M8, 2, P], fp8, tag="w8", name=f"w8_{mb}")
                    nc.sync.dma_start(out=w8_t[:], in_=w8_d[mb])
                i = 0 if mb < Q_SIZE // P else (1 if mb < (Q_SIZE + KV_SIZE) // P else 2)
                pss = [
                    ps_pool.tile([P, 512], f32, tag="ps", name=f"pm{mb}_{j}")
                    for j in range(NH)
                ]
                # fp8 DoubleRow k-slice (k-tiles 0..2*M8-1, from the pinned
                # x8 chunks), grouped at the head of the accumulation — DR
                # and normal matmuls must not alternate (each mode switch
                # costs ~45 ns of PE pipeline, measured)
                for q in range(M8):
                    x8_q = ab_tiles[q][0]
                    for j in range(NH):
                        nc.tensor.matmul(
                            pss[j][:],
                            w8_t[:, q, :, :],
                            x8_q[:, 0, :, j * 512:(j + 1) * 512],
                            start=(q == 0), stop=False,
                            perf_mode=DR,
                        )
                for kt in range(KBF):
                    for j in range(NH):
                        nc.tensor.matmul(
                            pss[j][:],
                            w_t[:, kt, :],
                            x_res[:, kt, j * 512:(j + 1) * 512],
                            start=False, stop=False,
                        )
                for j in range(NH):
                    nc.tensor.matmul(
                        pss[j][:],
                        b_t[:, mb, :],
                        ag[i][:, j * 512:(j + 1) * 512],
                        start=False, stop=True,
                    )
                st = st_pool.tile([P, TC], f32, tag="st", name=f"st{mb}")
                nc.vector.tensor_copy(st[:, 0:512], pss[0][:])
                nc.scalar.activation(st[:, 512:1024], pss[1][:], COPY)
                nc.scalar.dma_start(out=y_d[mb, :, 0:512], in_=st[:, 0:512])
                nc.scalar.dma_start(out=y_d[mb, :, 512:1024], in_=st[:, 512:1024])

    _elide_redundant_ldweights(nc, mybir)
    nc.compile()
    return nc


def _elide_redundant_ldweights(nc, mybir):
    """Drop an InstLdweights whose weights AP is identical to the previous
    load still sitting in the PE array (the j=0/j=1 n-half matmul pairs
    share their stationary). The PE keeps weights across matmuls, so the
    reload is pure overhead — for DoubleRow pairs it is the 213 ns
    bottleneck (256-col load, no FWL). Only sem-free loads are elided, so
    synchronization is untouched."""
    n_drop = 0
    for b in nc.m.functions[0].blocks:
        insts = b.instructions
        keep = []
        loaded_ap = None
        for i in insts:
            if isinstance(i, mybir.InstLdweights):
                ap = str(i.ins[0])
                si = i.sync_info
                clean = not si or (len(si.on_wait) == 0 and len(si.on_update) == 0)
                if clean and ap == loaded_ap:
                    n_drop += 1
                    continue
                loaded_ap = ap
            keep.append(i)
        if n_drop:
            insts.clear()
            insts.extend(keep)
    return n_drop


def _get_nc():
    if "nc" not in _CACHE:
        _CACHE["nc"] = _build_nc()
    return _CACHE["nc"]


def _prep_in_maps(x, W, lora_A, lora_B_q, lora_B_k, lora_B_v, scaling, token_to_slot):
    import ml_dtypes
    bf = ml_dtypes.bfloat16
    f8 = ml_dtypes.float8_e4m3fn
    f = np.float32
    x = np.asarray(x, dtype=f)
    W = np.asarray(W, dtype=f)

    # x moving operand, token-sharded: [c, p(k), kt, tl]  (t = c*1024 + tl)
    x_f = x.reshape(NCORES, TC, KA, P).transpose(0, 3, 2, 1)
    x_t = np.ascontiguousarray(x_f[:, :, 2 * M8:, :].astype(bf))
    x8_t = np.ascontiguousarray(
        (XS8 * x_f).reshape(NCORES, P, K2, 2, TC).astype(f8))
    # main GEMM stationary (replicated): [mb, p(k), kt, dl]  (d = mb*128 + dl)
    w_f = W.reshape(MB, P, KA, P).transpose(0, 3, 2, 1)
    w_t = np.ascontiguousarray(w_f[:, :, 2 * M8:, :].astype(bf))
    w8_t = np.ascontiguousarray(
        (WS8 * w_f[:, :, :2 * M8, :]).reshape(MB, P, M8, 2, P).astype(f8))
    # LoRA A stationary (fp8, pre-scaled): [p(k), kt2, ko, i, (s r)]
    a_f = (np.asarray(lora_A, dtype=f) * AS8).reshape(
        S, 3, R, KA, P).transpose(4, 3, 1, 0, 2).reshape(P, K2, 2, 3, S * R)
    a8_t = np.ascontiguousarray(a_f.astype(f8))
    # LoRA B stationary: [(s r), mb, dl]
    bq = np.asarray(lora_B_q, dtype=f).transpose(0, 2, 1).reshape(S * R, Q_SIZE)
    bk = np.asarray(lora_B_k, dtype=f).transpose(0, 2, 1).reshape(S * R, KV_SIZE)
    bv = np.asarray(lora_B_v, dtype=f).transpose(0, 2, 1).reshape(S * R, KV_SIZE)
    b_t = np.ascontiguousarray(
        np.concatenate([bq, bk, bv], axis=1).reshape(S * R, MB, P).astype(bf))
    # routing gate, expanded over ranks, with the fp8 x/A pre-scales folded
    # in: [c, (s r), tl]
    slot = np.asarray(token_to_slot).reshape(NCORES, TC)
    g = (slot[:, None, :] == np.arange(S, dtype=slot.dtype)[None, :, None])
    g = g.astype(f) * (np.asarray(scaling, dtype=f) / (AS8 * XS8))[None, :, None]
    gate = np.ascontiguousarray(np.repeat(g, R, axis=1))

    in_maps = []
    for c in range(NCORES):
        in_maps.append({
            "x_t": x_t[c],
            "x8_t": x8_t[c],
            "a8_t": a8_t,
            "w_t": w_t,
            "w8_t": w8_t,
            "b_t": b_t,
            "gate": gate[c],
        })
    return in_maps


def _assemble(results):
    # y[c] is [mb, dl, tl] fp32 — final values for core c's token shard
    return np.ascontiguousarray(np.concatenate(
        [results[c]["y"].reshape(D, TC).T for c in range(NCORES)], axis=0))


def _run(inputs, trace=False):
    from concourse.bass_utils import run_bass_kernel_spmd
    nc = _get_nc()
    in_maps = _prep_in_maps(**inputs)
    res = run_bass_kernel_spmd(
        nc, in_maps, core_ids=list(range(NCORES)), trace=trace)
    return res


def kernel(**inputs) -> np.ndarray:
    res = _run(inputs, trace=False)
    return _assemble(res.results)


if __name__ == "__main__":
    rng = np.random.default_rng(0)
    ins = {
        "x": rng.standard_normal((T, HID)).astype(np.float32),
        "W": (rng.standard_normal((D, HID)) * 0.02).astype(np.float32),
        "lora_A": (rng.standard_normal((S, 3, R, HID)) * 0.02).astype(np.float32),
        "lora_B_q": (rng.standard_normal((S, Q_SIZE, R)) * 0.02).astype(np.float32),
        "lora_B_k": (rng.standard_normal((S, KV_SIZE, R)) * 0.02).astype(np.float32),
        "lora_B_v": (rng.standard_normal((S, KV_SIZE, R)) * 0.02).astype(np.float32),
        "scaling": rng.uniform(0.5, 2.0, S).astype(np.float32),
        "token_to_slot": rng.integers(0, S, T).astype(np.int32),
    }
    out = kernel(**ins)
    print("out", out.shape, out.dtype)


# revision 47
# speedup vs baseline: 1.0055x; 1.0055x over previous
"""LoRA QKV fused projection kernel for 8 TRN2 NeuronCores.

Reference computation (T=8192 tokens, HID=4096, D=6144 out, S=8 slots, R=16):
    y = x @ W.T
    a[t,s,i,r] = sum_h x[t,h] * lora_A[s,i,r,h]         (down-proj, all slots)
    a *= onehot(token_to_slot)[t,s] * scaling[s]         (routing gate)
    d[t, :] = concat_i( sum_{s,r} a[t,s,i,r] * B_i[s,:,r] )   (up-proj)
    out = y + d

Sharding: pure token-DP — core c owns tokens [c*1024, (c+1)*1024) with the
full hidden and output dims. Its x shard (8.4 MB bf16) is loaded to SBUF once
as the main-GEMM moving operand; W streams through once (50 MB bf16/core).

Numerics/throughput choices (error budget: harness gate is 2e-2, we land
~1.68e-2, measured bit-stable across runs on the fixed-seed inputs):
  * main GEMM inputs bf16 (fp32 PSUM accumulate): bf16 enables the fast
    weight load path (fp32/fp32r cannot FWL), so the per-matmul stationary
    reload (~97 ns) hides under the previous matmul's 512-column stream in
    the PE's 64-deep reorder window. Measured back-to-back cadence: 216 ns.
  * LoRA down-proj AND 6 of the main GEMM's 32 k-tiles in fp8e4m3 with
    perf_mode=DoubleRow (2 k-tiles packed per PE cell, 2x MACs/cycle). A
    DoubleRow matmul costs ~213 ns (256-column LDWEIGHTS-bound, FWL off)
    but replaces TWO 216 ns bf16 matmuls. DR and bf16 matmuls are kept in
    contiguous blocks — alternating modes costs ~45 ns/switch (measured).
    Scale plan: x8 = x/4 and w8 = 4W (product exact in PSUM, no rescale
    hook exists); a8 = 16*lora_A with the 64x folded into the gate. The
    scales lift the 0.02-std weights out of e4m3's subnormal range.
  * LoRA up-proj fused into the main GEMM's PSUM accumulation: per output
    row-block mb, after the k-tile matmuls, one extra matmul with B as
    stationary (contracting the 128 (slot,rank) pairs of the gated
    down-proj activations) lands the delta in the same PSUM bank
    (start=False). Single drain emits final rows — no partials, no host
    reduce.

DMA plan: SP HWDGE ring carries w0,w1 -> x(bf16) -> w2..w47; ACT ring
carries the phase-A fp8 stream (x8/a8 chunks) -> gate -> B -> output drains.
"""

import numpy as np

# problem shape (hardcoded per harness contract)
T = 8192
HID = 4096
Q_SIZE = 4096
KV_SIZE = 1024
D = Q_SIZE + 2 * KV_SIZE  # 6144
S = 8
R = 16
NCORES = 8
P = 128

TC = T // NCORES          # 1024 tokens per core
KA = HID // P             # 32 k-tiles
K2 = KA // 2              # 16 double-row k-tile pairs
MB = D // P               # 48 output row-blocks of 128
NH = TC // 512            # 2 moving n-halves of 512 tokens
M8 = 3                    # k-tile PAIRS of the main GEMM done in fp8 DoubleRow
KBF = KA - 2 * M8         # remaining bf16 k-tiles of the main GEMM
# fp8 scale plan: products must come out exact in PSUM (no rescale hook), so
# scale(x8)*scale(W8) = 1; A's scale is compensated in the gate. The scales
# lift W (std 0.02) out of e4m3's subnormal range (<2^-6) while keeping x
# mostly normal.
XS8 = 0.25                # x8 = XS8 * x
AS8 = 16.0                # a8 = AS8 * lora_A
WS8 = 1.0 / XS8           # w8 = WS8 * W  (fp8 k-slice)

# phase-A chunk sizes in k-tile-pair units (tiny first chunks => first
# matmul waits on ~0.36 MB only). One SBUF buffer per chunk — a recycled
# buffer would make a later chunk's DMA wait on earlier matmuls, blocking
# the whole SP ring behind it (head-of-line).
CHUNKS2 = [1, 1, 1, 1, 2, 2, 2, 2, 2, 2]
assert sum(CHUNKS2) == K2
# bf16 x chunks on the SP ring (coarse; only consumed from phase C on; a
# smaller first chunk so mb0 can start sooner)
CHUNKS_X = [6, 10, 10]
assert sum(CHUNKS_X) == KBF

_CACHE = {}


def _build_nc():
    import concourse.mybir as mybir
    import concourse.tile as tile
    from concourse import bacc

    bf16 = mybir.dt.bfloat16
    fp8 = mybir.dt.float8e4
    f32 = mybir.dt.float32
    COPY = mybir.ActivationFunctionType.Copy
    DR = mybir.MatmulPerfMode.DoubleRow

    nc = bacc.Bacc(None, target_bir_lowering=False, debug=False)

    # ---- DRAM parameters (per-core shapes; declaration order = binding order)
    x_d = nc.declare_dram_parameter("x_t", [P, KBF, TC], bf16, isOutput=False)
    x8_d = nc.declare_dram_parameter("x8_t", [P, K2, 2, TC], fp8, isOutput=False)
    a8_d = nc.declare_dram_parameter("a8_t", [P, K2, 2, 3, P], fp8, isOutput=False)
    w_d = nc.declare_dram_parameter("w_t", [MB, P, KBF, P], bf16, isOutput=False)
    w8_d = nc.declare_dram_parameter("w8_t", [MB, P, M8, 2, P], fp8, isOutput=False)
    b_d = nc.declare_dram_parameter("b_t", [P, MB, P], bf16, isOutput=False)
    g_d = nc.declare_dram_parameter("gate", [P, TC], f32, isOutput=False)
    y_d = nc.declare_dram_parameter("y", [MB, P, TC], f32, isOutput=True)

    with tile.TileContext(nc) as tc:
        with tc.tile_pool(name="xres", bufs=1) as xres_pool, \
             tc.tile_pool(name="wp", bufs=3) as w_pool, \
             tc.tile_pool(name="x8p", bufs=10) as x8_pool, \
             tc.tile_pool(name="a8p", bufs=10) as a8_pool, \
             tc.tile_pool(name="agp", bufs=1) as ag_pool, \
             tc.tile_pool(name="bp", bufs=1) as b_pool, \
             tc.tile_pool(name="stp", bufs=4) as st_pool, \
             tc.tile_pool(name="psum", bufs=8, space="PSUM") as ps_pool:

            # resident bf16 moving operand for the main GEMM (k-tiles 4..31;
            # k-tiles 0..3 live in the pinned fp8 x8 chunks 0 and 1)
            x_res = xres_pool.tile([P, KBF, TC], bf16, tag="xres")

            # ---------------- Phase A: LoRA down-proj aT = A @ x (fp8) ------
            # aT[(i,sr), t] accumulated in 6 psum banks over 16 DoubleRow
            # k-pair steps, chasing the chunked x8/a8 loads. The fp8 stream
            # gets a clean prefix on the SP ring (critical early data); the
            # bulk bf16 x / W stream follows it.
            ps_a = [
                ps_pool.tile([P, 512], f32, tag="ps", name=f"ps_a{i}_{h}")
                for i in range(3) for h in range(2)
            ]
            k0s = [sum(CHUNKS2[:c]) for c in range(len(CHUNKS2))]
            ab_tiles = []
            for ch, (k0, cw) in enumerate(zip(k0s, CHUNKS2)):
                ksl = slice(k0, k0 + cw)
                x8_t = x8_pool.tile([P, cw, 2, TC], fp8, tag="x8", name=f"x8{ch}")
                nc.sync.dma_start(out=x8_t[:], in_=x8_d[:, ksl, :, :])
                a8_t = a8_pool.tile([P, cw, 2, 3, P], fp8, tag="a8", name=f"a8{ch}")
                nc.scalar.dma_start(out=a8_t[:], in_=a8_d[:, ksl, :, :, :])
                ab_tiles.append((x8_t, a8_t))
            # SP ring continues: x_res, then the W stream from mb2 on
            xk0 = 0
            for cw in CHUNKS_X:
                ksl = slice(xk0, xk0 + cw)
                nc.sync.dma_start(out=x_res[:, ksl, :], in_=x_d[:, ksl, :])
                xk0 += cw
            gate_t = ag_pool.tile([P, TC], f32, tag="gate")
            nc.scalar.dma_start(out=gate_t[:], in_=g_d[:])
            # first two W tiles ride the quiet ACT ring: the SP ring's front
            # (x8 + x_res) is the saturation bottleneck, and these aren't
            # needed until mb0/mb1 (~31/45 us)
            w_tiles = []
            for mb in range(2):
                w_t = w_pool.tile([P, KBF, P], bf16, tag="w", name=f"w{mb}")
                nc.scalar.dma_start(out=w_t[:], in_=w_d[mb])
                w8_t = w_pool.tile([P, M8, 2, P], fp8, tag="w8", name=f"w8_{mb}")
                nc.scalar.dma_start(out=w8_t[:], in_=w8_d[mb])
                w_tiles.append((w_t, w8_t))
            b_t = b_pool.tile([P, MB, P], bf16, tag="b")
            nc.scalar.dma_start(out=b_t[:], in_=b_d[:])

            for ch, (k0, cw) in enumerate(zip(k0s, CHUNKS2)):
                x8_t, a8_t = ab_tiles[ch]
                for kk in range(cw):
                    first = k0 + kk == 0
                    last = k0 + kk == K2 - 1
                    for i in range(3):
                        for h in range(2):
                            nc.tensor.matmul(
                                ps_a[i * 2 + h][:],
                                a8_t[:, kk, :, i, :],
                                x8_t[:, kk, :, h * 512:(h + 1) * 512],
                                start=first, stop=last,
                                perf_mode=DR,
                            )

            # ---------------- Phase B: routing gate -------------------------
            ag = []
            for i in range(3):
                ag_t = ag_pool.tile([P, TC], bf16, tag=f"ag{i}", name=f"ag{i}")
                for h in range(2):
                    sl = slice(h * 512, (h + 1) * 512)
                    nc.vector.tensor_mul(ag_t[:, sl], ps_a[i * 2 + h][:], gate_t[:, sl])
                ag.append(ag_t)

            # ---------------- Phase C: main GEMM + fused LoRA up-proj -------
            for mb in range(MB):
                if mb < 2:
                    w_t, w8_t = w_tiles[mb]
                else:
                    w_t = w_pool.tile([P, KBF, P], bf16, tag="w", name=f"w{mb}")
                    nc.sync.dma_start(out=w_t[:], in_=w_d[mb])
                    w8_t = w_pool.tile([P, M8, 2, P], fp8, tag="w8", name=f"w8_{mb}")
                    nc.sync.dma_start(out=w8_t[:], in_=w8_d[mb])
                i = 0 if mb < Q_SIZE // P else (1 if mb < (Q_SIZE + KV_SIZE) // P else 2)
                pss = [
                    ps_pool.tile([P, 512], f32, tag="ps", name=f"pm{mb}_{j}")
                    for j in range(NH)
                ]
                # fp8 DoubleRow k-slice (k-tiles 0..2*M8-1, from the pinned
                # x8 chunks), grouped at the head of the accumulation — DR
                # and normal matmuls must not alternate (each mode switch
                # costs ~45 ns of PE pipeline, measured)
                for q in range(M8):
                    x8_q = ab_tiles[q][0]
                    for j in range(NH):
                        nc.tensor.matmul(
                            pss[j][:],
                            w8_t[:, q, :, :],
                            x8_q[:, 0, :, j * 512:(j + 1) * 512],
                            start=(q == 0), stop=False,
                            perf_mode=DR,
                        )
                for kt in range(KBF):
                    for j in range(NH):
                        nc.tensor.matmul(
                            pss[j][:],
                            w_t[:, kt, :],
                            x_res[:, kt, j * 512:(j + 1) * 512],
                            start=False, stop=False,
                        )
                for j in range(NH):
                    nc.tensor.matmul(
                        pss[j][:],
                        b_t[:, mb, :],
                        ag[i][:, j * 512:(j + 1) * 512],
                        start=False, stop=True,
                    )
                st = st_pool.tile([P, TC], f32, tag="st", name=f"st{mb}")
                nc.vector.tensor_copy(st[:, 0:512], pss[0][:])
                nc.scalar.activation(st[:, 512:1024], pss[1][:], COPY)
                nc.scalar.dma_start(out=y_d[mb, :, 0:512], in_=st[:, 0:512])
                nc.scalar.dma_start(out=y_d[mb, :, 512:1024], in_=st[:, 512:1024])

    _elide_redundant_ldweights(nc, mybir)
    nc.compile()
    return nc


def _elide_redundant_ldweights(nc, mybir):
    """Drop an InstLdweights whose weights AP is identical to the previous
    load still sitting in the PE array (the j=0/j=1 n-half matmul pairs
    share their stationary). The PE keeps weights across matmuls, so the
    reload is pure overhead — for DoubleRow pairs it is the 213 ns
    bottleneck (256-col load, no FWL). Only sem-free loads are elided, so
    synchronization is untouched."""
    n_drop = 0
    for b in nc.m.functions[0].blocks:
        insts = b.instructions
        keep = []
        loaded_ap = None
        for i in insts:
            if isinstance(i, mybir.InstLdweights):
                ap = str(i.ins[0])
                si = i.sync_info
                clean = not si or (len(si.on_wait) == 0 and len(si.on_update) == 0)
                if clean and ap == loaded_ap:
                    n_drop += 1
                    continue
                loaded_ap = ap
            keep.append(i)
        if n_drop:
            insts.clear()
            insts.extend(keep)
    return n_drop


def _get_nc():
    if "nc" not in _CACHE:
        _CACHE["nc"] = _build_nc()
    return _CACHE["nc"]


def _prep_in_maps(x, W, lora_A, lora_B_q, lora_B_k, lora_B_v, scaling, token_to_slot):
    import ml_dtypes
    bf = ml_dtypes.bfloat16
    f8 = ml_dtypes.float8_e4m3fn
    f = np.float32
    x = np.asarray(x, dtype=f)
    W = np.asarray(W, dtype=f)

    # x moving operand, token-sharded: [c, p(k), kt, tl]  (t = c*1024 + tl)
    x_f = x.reshape(NCORES, TC, KA, P).transpose(0, 3, 2, 1)
    x_t = np.ascontiguousarray(x_f[:, :, 2 * M8:, :].astype(bf))
    x8_t = np.ascontiguousarray(
        (XS8 * x_f).reshape(NCORES, P, K2, 2, TC).astype(f8))
    # main GEMM stationary (replicated): [mb, p(k), kt, dl]  (d = mb*128 + dl)
    w_f = W.reshape(MB, P, KA, P).transpose(0, 3, 2, 1)
    w_t = np.ascontiguousarray(w_f[:, :, 2 * M8:, :].astype(bf))
    w8_t = np.ascontiguousarray(
        (WS8 * w_f[:, :, :2 * M8, :]).reshape(MB, P, M8, 2, P).astype(f8))
    # LoRA A stationary (fp8, pre-scaled): [p(k), kt2, ko, i, (s r)]
    a_f = (np.asarray(lora_A, dtype=f) * AS8).reshape(
        S, 3, R, KA, P).transpose(4, 3, 1, 0, 2).reshape(P, K2, 2, 3, S * R)
    a8_t = np.ascontiguousarray(a_f.astype(f8))
    # LoRA B stationary: [(s r), mb, dl]
    bq = np.asarray(lora_B_q, dtype=f).transpose(0, 2, 1).reshape(S * R, Q_SIZE)
    bk = np.asarray(lora_B_k, dtype=f).transpose(0, 2, 1).reshape(S * R, KV_SIZE)
    bv = np.asarray(lora_B_v, dtype=f).transpose(0, 2, 1).reshape(S * R, KV_SIZE)
    b_t = np.ascontiguousarray(
        np.concatenate([bq, bk, bv], axis=1).reshape(S * R, MB, P).astype(bf))
    # routing gate, expanded over ranks, with the fp8 x/A pre-scales folded
    # in: [c, (s r), tl]
    slot = np.asarray(token_to_slot).reshape(NCORES, TC)
    g = (slot[:, None, :] == np.arange(S, dtype=slot.dtype)[None, :, None])
    g = g.astype(f) * (np.asarray(scaling, dtype=f) / (AS8 * XS8))[None, :, None]
    gate = np.ascontiguousarray(np.repeat(g, R, axis=1))

    in_maps = []
    for c in range(NCORES):
        in_maps.append({
            "x_t": x_t[c],
            "x8_t": x8_t[c],
            "a8_t": a8_t,
            "w_t": w_t,
            "w8_t": w8_t,
            "b_t": b_t,
            "gate": gate[c],
        })
    return in_maps


def _assemble(results):
    # y[c] is [mb, dl, tl] fp32 — final values for core c's token shard
    return np.ascontiguousarray(np.concatenate(
        [results[c]["y"].reshape(D, TC).T for c in range(NCORES)], axis=0))


def _run(inputs, trace=False):
    from concourse.bass_utils import run_bass_kernel_spmd
    nc = _get_nc()
    in_maps = _prep_in_maps(**inputs)
    res = run_bass_kernel_spmd(
        nc, in_maps, core_ids=list(range(NCORES)), trace=trace)
    return res


def kernel(**inputs) -> np.ndarray:
    res = _run(inputs, trace=False)
    return _assemble(res.results)


if __name__ == "__main__":
    rng = np.random.default_rng(0)
    ins = {
        "x": rng.standard_normal((T, HID)).astype(np.float32),
        "W": (rng.standard_normal((D, HID)) * 0.02).astype(np.float32),
        "lora_A": (rng.standard_normal((S, 3, R, HID)) * 0.02).astype(np.float32),
        "lora_B_q": (rng.standard_normal((S, Q_SIZE, R)) * 0.02).astype(np.float32),
        "lora_B_k": (rng.standard_normal((S, KV_SIZE, R)) * 0.02).astype(np.float32),
        "lora_B_v": (rng.standard_normal((S, KV_SIZE, R)) * 0.02).astype(np.float32),
        "scaling": rng.uniform(0.5, 2.0, S).astype(np.float32),
        "token_to_slot": rng.integers(0, S, T).astype(np.int32),
    }
    out = kernel(**ins)
    print("out", out.shape, out.dtype)


# revision 48
# speedup vs baseline: 1.0136x; 1.0081x over previous
"""LoRA QKV fused projection kernel for 8 TRN2 NeuronCores.

Reference computation (T=8192 tokens, HID=4096, D=6144 out, S=8 slots, R=16):
    y = x @ W.T
    a[t,s,i,r] = sum_h x[t,h] * lora_A[s,i,r,h]         (down-proj, all slots)
    a *= onehot(token_to_slot)[t,s] * scaling[s]         (routing gate)
    d[t, :] = concat_i( sum_{s,r} a[t,s,i,r] * B_i[s,:,r] )   (up-proj)
    out = y + d

Sharding: pure token-DP — core c owns tokens [c*1024, (c+1)*1024) with the
full hidden and output dims. Its x shard (8.4 MB bf16) is loaded to SBUF once
as the main-GEMM moving operand; W streams through once (50 MB bf16/core).

Numerics/throughput choices (error budget: harness gate is 2e-2, we land
~1.68e-2, measured bit-stable across runs on the fixed-seed inputs):
  * main GEMM inputs bf16 (fp32 PSUM accumulate): bf16 enables the fast
    weight load path (fp32/fp32r cannot FWL), so the per-matmul stationary
    reload (~97 ns) hides under the previous matmul's 512-column stream in
    the PE's 64-deep reorder window. Measured back-to-back cadence: 216 ns.
  * LoRA down-proj AND 6 of the main GEMM's 32 k-tiles in fp8e4m3 with
    perf_mode=DoubleRow (2 k-tiles packed per PE cell, 2x MACs/cycle). A
    DoubleRow matmul costs ~213 ns (256-column LDWEIGHTS-bound, FWL off)
    but replaces TWO 216 ns bf16 matmuls. DR and bf16 matmuls are kept in
    contiguous blocks — alternating modes costs ~45 ns/switch (measured).
    Scale plan: x8 = x/4 and w8 = 4W (product exact in PSUM, no rescale
    hook exists); a8 = 16*lora_A with the 64x folded into the gate. The
    scales lift the 0.02-std weights out of e4m3's subnormal range.
  * LoRA up-proj fused into the main GEMM's PSUM accumulation: per output
    row-block mb, after the k-tile matmuls, one extra matmul with B as
    stationary (contracting the 128 (slot,rank) pairs of the gated
    down-proj activations) lands the delta in the same PSUM bank
    (start=False). Single drain emits final rows — no partials, no host
    reduce.

DMA plan: SP HWDGE ring carries w0,w1 -> x(bf16) -> w2..w47; ACT ring
carries the phase-A fp8 stream (x8/a8 chunks) -> gate -> B -> output drains.
"""

import numpy as np

# problem shape (hardcoded per harness contract)
T = 8192
HID = 4096
Q_SIZE = 4096
KV_SIZE = 1024
D = Q_SIZE + 2 * KV_SIZE  # 6144
S = 8
R = 16
NCORES = 8
P = 128

TC = T // NCORES          # 1024 tokens per core
KA = HID // P             # 32 k-tiles
K2 = KA // 2              # 16 double-row k-tile pairs
MB = D // P               # 48 output row-blocks of 128
NH = TC // 512            # 2 moving n-halves of 512 tokens
M8 = 3                    # k-tile PAIRS of the main GEMM done in fp8 DoubleRow
KBF = KA - 2 * M8         # remaining bf16 k-tiles of the main GEMM
# fp8 scale plan: products must come out exact in PSUM (no rescale hook), so
# scale(x8)*scale(W8) = 1; A's scale is compensated in the gate. The scales
# lift W (std 0.02) out of e4m3's subnormal range (<2^-6) while keeping x
# mostly normal.
XS8 = 0.25                # x8 = XS8 * x
AS8 = 16.0                # a8 = AS8 * lora_A
WS8 = 1.0 / XS8           # w8 = WS8 * W  (fp8 k-slice)

# phase-A chunk sizes in k-tile-pair units (tiny first chunks => first
# matmul waits on ~0.36 MB only). One SBUF buffer per chunk — a recycled
# buffer would make a later chunk's DMA wait on earlier matmuls, blocking
# the whole SP ring behind it (head-of-line).
CHUNKS2 = [1, 1, 1, 1, 2, 2, 2, 2, 2, 2]
assert sum(CHUNKS2) == K2
# bf16 x chunks on the SP ring (coarse; only consumed from phase C on; a
# smaller first chunk so mb0 can start sooner)
CHUNKS_X = [6, 5, 5, 5, 5]
assert sum(CHUNKS_X) == KBF

_CACHE = {}


def _build_nc():
    import concourse.mybir as mybir
    import concourse.tile as tile
    from concourse import bacc

    bf16 = mybir.dt.bfloat16
    fp8 = mybir.dt.float8e4
    f32 = mybir.dt.float32
    COPY = mybir.ActivationFunctionType.Copy
    DR = mybir.MatmulPerfMode.DoubleRow

    nc = bacc.Bacc(None, target_bir_lowering=False, debug=False)

    # ---- DRAM parameters (per-core shapes; declaration order = binding order)
    x_d = nc.declare_dram_parameter("x_t", [P, KBF, TC], bf16, isOutput=False)
    x8_d = nc.declare_dram_parameter("x8_t", [P, K2, 2, TC], fp8, isOutput=False)
    a8_d = nc.declare_dram_parameter("a8_t", [P, K2, 2, 3, P], fp8, isOutput=False)
    w_d = nc.declare_dram_parameter("w_t", [MB, P, KBF, P], bf16, isOutput=False)
    w8_d = nc.declare_dram_parameter("w8_t", [MB, P, M8, 2, P], fp8, isOutput=False)
    b_d = nc.declare_dram_parameter("b_t", [P, MB, P], bf16, isOutput=False)
    g_d = nc.declare_dram_parameter("gate", [P, TC], f32, isOutput=False)
    y_d = nc.declare_dram_parameter("y", [MB, P, TC], f32, isOutput=True)

    with tile.TileContext(nc) as tc:
        with tc.tile_pool(name="xres", bufs=1) as xres_pool, \
             tc.tile_pool(name="wp", bufs=3) as w_pool, \
             tc.tile_pool(name="x8p", bufs=10) as x8_pool, \
             tc.tile_pool(name="a8p", bufs=10) as a8_pool, \
             tc.tile_pool(name="agp", bufs=1) as ag_pool, \
             tc.tile_pool(name="bp", bufs=1) as b_pool, \
             tc.tile_pool(name="stp", bufs=4) as st_pool, \
             tc.tile_pool(name="psum", bufs=8, space="PSUM") as ps_pool:

            # resident bf16 moving operand for the main GEMM (k-tiles 4..31;
            # k-tiles 0..3 live in the pinned fp8 x8 chunks 0 and 1)
            x_res = xres_pool.tile([P, KBF, TC], bf16, tag="xres")

            # ---------------- Phase A: LoRA down-proj aT = A @ x (fp8) ------
            # aT[(i,sr), t] accumulated in 6 psum banks over 16 DoubleRow
            # k-pair steps, chasing the chunked x8/a8 loads. The fp8 stream
            # gets a clean prefix on the SP ring (critical early data); the
            # bulk bf16 x / W stream follows it.
            ps_a = [
                ps_pool.tile([P, 512], f32, tag="ps", name=f"ps_a{i}_{h}")
                for i in range(3) for h in range(2)
            ]
            k0s = [sum(CHUNKS2[:c]) for c in range(len(CHUNKS2))]
            ab_tiles = []
            for ch, (k0, cw) in enumerate(zip(k0s, CHUNKS2)):
                ksl = slice(k0, k0 + cw)
                x8_t = x8_pool.tile([P, cw, 2, TC], fp8, tag="x8", name=f"x8{ch}")
                nc.sync.dma_start(out=x8_t[:], in_=x8_d[:, ksl, :, :])
                a8_t = a8_pool.tile([P, cw, 2, 3, P], fp8, tag="a8", name=f"a8{ch}")
                nc.scalar.dma_start(out=a8_t[:], in_=a8_d[:, ksl, :, :, :])
                ab_tiles.append((x8_t, a8_t))
            # SP ring continues: x_res, then the W stream from mb2 on
            xk0 = 0
            for cw in CHUNKS_X:
                ksl = slice(xk0, xk0 + cw)
                nc.sync.dma_start(out=x_res[:, ksl, :], in_=x_d[:, ksl, :])
                xk0 += cw
            gate_t = ag_pool.tile([P, TC], f32, tag="gate")
            nc.scalar.dma_start(out=gate_t[:], in_=g_d[:])
            # first two W tiles ride the quiet ACT ring: the SP ring's front
            # (x8 + x_res) is the saturation bottleneck, and these aren't
            # needed until mb0/mb1 (~31/45 us)
            w_tiles = []
            for mb in range(2):
                w_t = w_pool.tile([P, KBF, P], bf16, tag="w", name=f"w{mb}")
                nc.scalar.dma_start(out=w_t[:], in_=w_d[mb])
                w8_t = w_pool.tile([P, M8, 2, P], fp8, tag="w8", name=f"w8_{mb}")
                nc.scalar.dma_start(out=w8_t[:], in_=w8_d[mb])
                w_tiles.append((w_t, w8_t))
            b_t = b_pool.tile([P, MB, P], bf16, tag="b")
            nc.scalar.dma_start(out=b_t[:], in_=b_d[:])

            for ch, (k0, cw) in enumerate(zip(k0s, CHUNKS2)):
                x8_t, a8_t = ab_tiles[ch]
                for kk in range(cw):
                    first = k0 + kk == 0
                    last = k0 + kk == K2 - 1
                    for i in range(3):
                        for h in range(2):
                            nc.tensor.matmul(
                                ps_a[i * 2 + h][:],
                                a8_t[:, kk, :, i, :],
                                x8_t[:, kk, :, h * 512:(h + 1) * 512],
                                start=first, stop=last,
                                perf_mode=DR,
                            )

            # ---------------- Phase B: routing gate -------------------------
            ag = []
            for i in range(3):
                ag_t = ag_pool.tile([P, TC], bf16, tag=f"ag{i}", name=f"ag{i}")
                for h in range(2):
                    sl = slice(h * 512, (h + 1) * 512)
                    nc.vector.tensor_mul(ag_t[:, sl], ps_a[i * 2 + h][:], gate_t[:, sl])
                ag.append(ag_t)

            # ---------------- Phase C: main GEMM + fused LoRA up-proj -------
            for mb in range(MB):
                if mb < 2:
                    w_t, w8_t = w_tiles[mb]
                else:
                    w_t = w_pool.tile([P, KBF, P], bf16, tag="w", name=f"w{mb}")
                    nc.sync.dma_start(out=w_t[:], in_=w_d[mb])
                    w8_t = w_pool.tile([P, M8, 2, P], fp8, tag="w8", name=f"w8_{mb}")
                    nc.sync.dma_start(out=w8_t[:], in_=w8_d[mb])
                i = 0 if mb < Q_SIZE // P else (1 if mb < (Q_SIZE + KV_SIZE) // P else 2)
                pss = [
                    ps_pool.tile([P, 512], f32, tag="ps", name=f"pm{mb}_{j}")
                    for j in range(NH)
                ]
                # fp8 DoubleRow k-slice (k-tiles 0..2*M8-1, from the pinned
                # x8 chunks), grouped at the head of the accumulation — DR
                # and normal matmuls must not alternate (each mode switch
                # costs ~45 ns of PE pipeline, measured)
                for q in range(M8):
                    x8_q = ab_tiles[q][0]
                    for j in range(NH):
                        nc.tensor.matmul(
                            pss[j][:],
                            w8_t[:, q, :, :],
                            x8_q[:, 0, :, j * 512:(j + 1) * 512],
                            start=(q == 0), stop=False,
                            perf_mode=DR,
                        )
                for kt in range(KBF):
                    for j in range(NH):
                        nc.tensor.matmul(
                            pss[j][:],
                            w_t[:, kt, :],
                            x_res[:, kt, j * 512:(j + 1) * 512],
                            start=False, stop=False,
                        )
                for j in range(NH):
                    nc.tensor.matmul(
                        pss[j][:],
                        b_t[:, mb, :],
                        ag[i][:, j * 512:(j + 1) * 512],
                        start=False, stop=True,
                    )
                st = st_pool.tile([P, TC], f32, tag="st", name=f"st{mb}")
                nc.vector.tensor_copy(st[:, 0:512], pss[0][:])
                nc.scalar.activation(st[:, 512:1024], pss[1][:], COPY)
                nc.scalar.dma_start(out=y_d[mb, :, 0:512], in_=st[:, 0:512])
                nc.scalar.dma_start(out=y_d[mb, :, 512:1024], in_=st[:, 512:1024])

    _elide_redundant_ldweights(nc, mybir)
    nc.compile()
    return nc


def _elide_redundant_ldweights(nc, mybir):
    """Drop an InstLdweights whose weights AP is identical to the previous
    load still sitting in the PE array (the j=0/j=1 n-half matmul pairs
    share their stationary). The PE keeps weights across matmuls, so the
    reload is pure overhead — for DoubleRow pairs it is the 213 ns
    bottleneck (256-col load, no FWL). Only sem-free loads are elided, so
    synchronization is untouched."""
    n_drop = 0
    for b in nc.m.functions[0].blocks:
        insts = b.instructions
        keep = []
        loaded_ap = None
        for i in insts:
            if isinstance(i, mybir.InstLdweights):
                ap = str(i.ins[0])
                si = i.sync_info
                clean = not si or (len(si.on_wait) == 0 and len(si.on_update) == 0)
                if clean and ap == loaded_ap:
                    n_drop += 1
                    continue
                loaded_ap = ap
            keep.append(i)
        if n_drop:
            insts.clear()
            insts.extend(keep)
    return n_drop


def _get_nc():
    if "nc" not in _CACHE:
        _CACHE["nc"] = _build_nc()
    return _CACHE["nc"]


def _prep_in_maps(x, W, lora_A, lora_B_q, lora_B_k, lora_B_v, scaling, token_to_slot):
    import ml_dtypes
    bf = ml_dtypes.bfloat16
    f8 = ml_dtypes.float8_e4m3fn
    f = np.float32
    x = np.asarray(x, dtype=f)
    W = np.asarray(W, dtype=f)

    # x moving operand, token-sharded: [c, p(k), kt, tl]  (t = c*1024 + tl)
    x_f = x.reshape(NCORES, TC, KA, P).transpose(0, 3, 2, 1)
    x_t = np.ascontiguousarray(x_f[:, :, 2 * M8:, :].astype(bf))
    x8_t = np.ascontiguousarray(
        (XS8 * x_f).reshape(NCORES, P, K2, 2, TC).astype(f8))
    # main GEMM stationary (replicated): [mb, p(k), kt, dl]  (d = mb*128 + dl)
    w_f = W.reshape(MB, P, KA, P).transpose(0, 3, 2, 1)
    w_t = np.ascontiguousarray(w_f[:, :, 2 * M8:, :].astype(bf))
    w8_t = np.ascontiguousarray(
        (WS8 * w_f[:, :, :2 * M8, :]).reshape(MB, P, M8, 2, P).astype(f8))
    # LoRA A stationary (fp8, pre-scaled): [p(k), kt2, ko, i, (s r)]
    a_f = (np.asarray(lora_A, dtype=f) * AS8).reshape(
        S, 3, R, KA, P).transpose(4, 3, 1, 0, 2).reshape(P, K2, 2, 3, S * R)
    a8_t = np.ascontiguousarray(a_f.astype(f8))
    # LoRA B stationary: [(s r), mb, dl]
    bq = np.asarray(lora_B_q, dtype=f).transpose(0, 2, 1).reshape(S * R, Q_SIZE)
    bk = np.asarray(lora_B_k, dtype=f).transpose(0, 2, 1).reshape(S * R, KV_SIZE)
    bv = np.asarray(lora_B_v, dtype=f).transpose(0, 2, 1).reshape(S * R, KV_SIZE)
    b_t = np.ascontiguousarray(
        np.concatenate([bq, bk, bv], axis=1).reshape(S * R, MB, P).astype(bf))
    # routing gate, expanded over ranks, with the fp8 x/A pre-scales folded
    # in: [c, (s r), tl]
    slot = np.asarray(token_to_slot).reshape(NCORES, TC)
    g = (slot[:, None, :] == np.arange(S, dtype=slot.dtype)[None, :, None])
    g = g.astype(f) * (np.asarray(scaling, dtype=f) / (AS8 * XS8))[None, :, None]
    gate = np.ascontiguousarray(np.repeat(g, R, axis=1))

    in_maps = []
    for c in range(NCORES):
        in_maps.append({
            "x_t": x_t[c],
            "x8_t": x8_t[c],
            "a8_t": a8_t,
            "w_t": w_t,
            "w8_t": w8_t,
            "b_t": b_t,
            "gate": gate[c],
        })
    return in_maps


def _assemble(results):
    # y[c] is [mb, dl, tl] fp32 — final values for core c's token shard
    return np.ascontiguousarray(np.concatenate(
        [results[c]["y"].reshape(D, TC).T for c in range(NCORES)], axis=0))


def _run(inputs, trace=False):
    from concourse.bass_utils import run_bass_kernel_spmd
    nc = _get_nc()
    in_maps = _prep_in_maps(**inputs)
    res = run_bass_kernel_spmd(
        nc, in_maps, core_ids=list(range(NCORES)), trace=trace)
    return res


def kernel(**inputs) -> np.ndarray:
    res = _run(inputs, trace=False)
    return _assemble(res.results)


if __name__ == "__main__":
    rng = np.random.default_rng(0)
    ins = {
        "x": rng.standard_normal((T, HID)).astype(np.float32),
        "W": (rng.standard_normal((D, HID)) * 0.02).astype(np.float32),
        "lora_A": (rng.standard_normal((S, 3, R, HID)) * 0.02).astype(np.float32),
        "lora_B_q": (rng.standard_normal((S, Q_SIZE, R)) * 0.02).astype(np.float32),
        "lora_B_k": (rng.standard_normal((S, KV_SIZE, R)) * 0.02).astype(np.float32),
        "lora_B_v": (rng.standard_normal((S, KV_SIZE, R)) * 0.02).astype(np.float32),
        "scaling": rng.uniform(0.5, 2.0, S).astype(np.float32),
        "token_to_slot": rng.integers(0, S, T).astype(np.int32),
    }
    out = kernel(**ins)
    print("out", out.shape, out.dtype)
